# revision 16
# baseline (speedup 1.0000x reference)
"""Multi-head attention (B=2, T=4096, C=768, H=12, Dk=64) on 8 trn2 NeuronCores.

Sharding: core c -> batch b = c//4, head-group g = c%4 (3 heads each).
Megatron-style: each core computes qkv projection for its 3 heads, full
attention for those heads, and a row-parallel partial of the output
projection. Host sums the 4 partials per batch (+ bias, folded into the
g==0 core's partial on device).

Device algorithm (per core), everything fp32:
  - qkT[c, t] feature-major via matmul(lhsT=Wqk_cols, rhs=xT) with
    column packing [q0 q1 | k0 k1 | q2 k2] so head0 lives on SBUF
    partitions 0-63 and head1 on 64-127 (natural PE row-tiling pairs),
    plus a swapped duplicate slot o3 = [k2 | q2] so head2 pairs across
    alternating tk-blocks.
  - V token-major [t, 64] per head with an appended ones column ->
    attention matmul also accumulates the softmax denominator.
  - attention in S^T layout: ST[tk,tq-blk] = KT^T@QT, exp on ACT engine
    (scale=1/8 fused into the activation), OT^T[dv,tq] += V_aug^T@expST.
  - normalize with DVE reciprocal + gpsimd partition broadcast.
  - out projection from OT^T (feature-major) with Wout rows.
"""

import os
import sys
from contextlib import ExitStack

import numpy as np

for _p in ("/opt/trn_rl_repo", "/root/.axon_site/_ro/trn_rl_repo"):
    if os.path.isdir(_p) and _p not in sys.path:
        sys.path.append(_p)

import concourse.bass as bass
import concourse.mybir as mybir
import concourse.tile as tile
from concourse import bacc
from concourse.bass import ts
from concourse.bass_utils import run_bass_kernel_spmd

F32 = mybir.dt.float32
B, T, C = 2, 4096, 768
H, DK = 12, 64
N_CORES = 8
HPC = 3  # heads per core
GQ = 512  # q-block (matmul free dim)
NTQ = T // GQ  # 8 q-blocks
NTK = T // 128  # 32 tk-blocks


def _build_program(debug_taps=False):
    nc = bacc.Bacc("TRN2", target_bir_lowering=False, debug=False)

    xT = nc.dram_tensor("xT", [C, T], F32, kind="ExternalInput").ap()
    wqk = nc.dram_tensor("wqk", [C, 384], F32, kind="ExternalInput").ap()
    bqk = nc.dram_tensor("bqk", [384], F32, kind="ExternalInput").ap()
    wv = nc.dram_tensor("wv", [C, 192], F32, kind="ExternalInput").ap()
    bv = nc.dram_tensor("bv", [192], F32, kind="ExternalInput").ap()
    wout = nc.dram_tensor("wout", [192, C], F32, kind="ExternalInput").ap()
    bout = nc.dram_tensor("bout", [C], F32, kind="ExternalInput").ap()
    y = nc.dram_tensor("y", [T, C], F32, kind="ExternalOutput").ap()

    xT3 = xT.rearrange("(o p) t -> p o t", p=128)  # [128, 6, 4096]
    y3 = y.rearrange("(n p) e -> p n e", p=128)  # [128, 32, 768]

    dbg = {}
    if debug_taps:
        for name, shape in [
            ("dbg_qkT", [128, 4, 512]),
            ("dbg_vaug", [128, 3, 65]),
            ("dbg_est", [128, 2, 512]),
            ("dbg_ot", [65, 512]),
            ("dbg_rc", [1, 512]),
            ("dbg_rb", [64, 512]),
            ("dbg_otn", [64, 512]),
        ]:
            dbg[name] = nc.dram_tensor(name, shape, F32, kind="ExternalOutput").ap()

    with tile.TileContext(nc) as tc, ExitStack() as ctx:
        sb = ctx.enter_context(tc.tile_pool(name="persist", bufs=1))

        # --- weights / biases ---
        wqk_sb = sb.tile([128, 6, 384], F32)
        nc.sync.dma_start(wqk_sb[:], wqk.rearrange("(o p) c -> p o c", p=128))
        wv_sb = sb.tile([128, 6, 192], F32)
        nc.sync.dma_start(wv_sb[:], wv.rearrange("(o p) c -> p o c", p=128))
        wout_sb = sb.tile([64, 3, C], F32)
        nc.sync.dma_start(wout_sb[:], wout.rearrange("(h p) e -> p h e", p=64))
        bqk_sb = sb.tile([128, 3], F32)
        nc.sync.dma_start(bqk_sb[:], bqk.rearrange("(o p) -> p o", p=128))
        bv_bc = sb.tile([128, 192], F32)
        nc.sync.dma_start(bv_bc[:], bv[None, :].to_broadcast((128, 192)))
        bout_bc = sb.tile([128, C], F32)
        nc.sync.dma_start(bout_bc[:], bout[None, :].to_broadcast((128, C)))

        # --- persistent activations ---
        # qkT slots: o0=[q0|q1] o1=[k0|k1] o2=[q2|k2] o3=[k2|q2]
        qkT = sb.tile([128, 4, T], F32)
        # V per tk-block per head, token-major, with ones col at [.., 64]
        vaug = sb.tile([128, NTK, HPC, 65], F32)
        nc.gpsimd.memset(vaug[:, :, :, 64:65], 1.0)

        # --- prologue: qkv projections ---
        with (
            tc.tile_pool(name="psA", bufs=2, space="PSUM") as psA,
            tc.tile_pool(name="xin", bufs=2) as xin,
        ):
            for tb in range(NTQ):  # qk proj over 512-col blocks of t
                xt = xin.tile([128, 6, GQ], F32, tag="xqk")
                nc.sync.dma_start(xt[:], xT3[:, :, ts(tb, GQ)])
                for cb in range(3):
                    ps = psA.tile([128, GQ], F32, tag="qk")
                    for d in range(6):
                        nc.tensor.matmul(
                            ps[:],
                            wqk_sb[:, d, ts(cb, 128)],
                            xt[:, d, :],
                            start=(d == 0),
                            stop=(d == 5),
                        )
                    nc.vector.tensor_scalar_add(
                        qkT[:, cb, ts(tb, GQ)], ps[:], bqk_sb[:, cb : cb + 1]
                    )
                # o3 = swap halves of o2 (k2|q2)
                nc.sync.dma_start(qkT[0:64, 3, ts(tb, GQ)], qkT[64:128, 2, ts(tb, GQ)])
                nc.sync.dma_start(qkT[64:128, 3, ts(tb, GQ)], qkT[0:64, 2, ts(tb, GQ)])

            for tb in range(NTK):  # v proj over 128-row blocks of t
                xv = xin.tile([128, 6, 128], F32, tag="xv")
                nc.sync.dma_start(xv[:], xT3[:, :, ts(tb, 128)])
                psv = psA.tile([128, 192], F32, tag="v")
                for d in range(6):
                    nc.tensor.matmul(
                        psv[:],
                        xv[:, d, :],
                        wv_sb[:, d, :],
                        start=(d == 0),
                        stop=(d == 5),
                    )
                nc.vector.tensor_tensor(
                    vaug[:, tb, :, 0:64],
                    psv[:].rearrange("p (h d) -> p h d", d=64),
                    bv_bc[:].rearrange("p (h d) -> p h d", d=64),
                    mybir.AluOpType.add,
                )

        # --- attention + output projection ---
        with (
            tc.tile_pool(name="psST", bufs=2, space="PSUM") as psST,
            tc.tile_pool(name="psOT", bufs=3, space="PSUM") as psOT,
            tc.tile_pool(name="psY", bufs=1, space="PSUM") as psY,
            tc.tile_pool(name="estp", bufs=3) as estp,
            tc.tile_pool(name="otp", bufs=2) as otp,
            tc.tile_pool(name="smallp", bufs=3) as smallp,
            tc.tile_pool(name="yp", bufs=2) as yp,
            tc.tile_pool(name="dramp", bufs=3, space="DRAM") as dramp,
        ):

            def normalize(ps_ot, ot_dst, tap=False):
                rc = smallp.tile([1, GQ], F32, tag="rc")
                nc.vector.reciprocal(rc[:], ps_ot[64:65, :])
                dn = dramp.tile([GQ], F32, tag="dn")
                nc.sync.dma_start(dn[:], rc[:])
                rb = smallp.tile([64, GQ], F32, tag="rb")
                nc.sync.dma_start(rb[:], dn[None, :].to_broadcast((64, GQ)))
                nc.vector.tensor_tensor(
                    ot_dst, ps_ot[0:64, :], rb[:], mybir.AluOpType.mult
                )
                if tap:
                    ots = smallp.tile([65, GQ], F32, tag="dbg_ots")
                    nc.vector.tensor_copy(ots[:], ps_ot[:])
                    nc.sync.dma_start(dbg["dbg_ot"], ots[:])
                    nc.sync.dma_start(dbg["dbg_rc"], rc[:])
                    nc.sync.dma_start(dbg["dbg_rb"], rb[:])
                    nc.sync.dma_start(dbg["dbg_otn"], ot_dst)

            for tq in range(NTQ):
                ot_tile = otp.tile([64, HPC, GQ], F32, tag="ot_sb")

                # -- heads 0,1 (paired on PE rows lo/hi) --
                ps_ot0 = psOT.tile([65, GQ], F32, tag="ot")
                ps_ot1 = psOT.tile([65, GQ], F32, tag="ot")
                for g in range(NTK):
                    st = psST.tile([128, 2, GQ], F32, tag="st")
                    nc.tensor.matmul(
                        st[:, 0, :],
                        qkT[0:64, 1, ts(g, 128)],
                        qkT[0:64, 0, ts(tq, GQ)],
                        start=True,
                        stop=True,
                    )
                    nc.tensor.matmul(
                        st[:, 1, :],
                        qkT[64:128, 1, ts(g, 128)],
                        qkT[64:128, 0, ts(tq, GQ)],
                        start=True,
                        stop=True,
                    )
                    est = estp.tile([128, 2, GQ], F32, tag="est")
                    nc.scalar.activation(
                        est[:], st[:], mybir.ActivationFunctionType.Exp, scale=0.125
                    )
                    if debug_taps and tq == 0 and g == 0:
                        nc.sync.dma_start(dbg["dbg_est"], est[:])
                        nc.sync.dma_start(dbg["dbg_qkT"], qkT[:, :, 0:512])
                        nc.sync.dma_start(dbg["dbg_vaug"], vaug[:, 0, :, :])
                    nc.tensor.matmul(
                        ps_ot0[:],
                        vaug[:, g, 0, :],
                        est[:, 0, :],
                        start=(g == 0),
                        stop=(g == NTK - 1),
                    )
                    nc.tensor.matmul(
                        ps_ot1[:],
                        vaug[:, g, 1, :],
                        est[:, 1, :],
                        start=(g == 0),
                        stop=(g == NTK - 1),
                    )
                normalize(ps_ot0, ot_tile[:, 0, :], tap=(debug_taps and tq == 0))
                normalize(ps_ot1, ot_tile[:, 1, :])

                # -- head 2 (paired across even/odd tk-blocks) --
                ps_ot2 = psOT.tile([65, GQ], F32, tag="ot")
                for g2 in range(NTK // 2):
                    ge, go = 2 * g2, 2 * g2 + 1
                    st = psST.tile([128, 2, GQ], F32, tag="st")
                    nc.tensor.matmul(
                        st[:, 0, :],
                        qkT[0:64, 3, ts(ge, 128)],
                        qkT[0:64, 2, ts(tq, GQ)],
                        start=True,
                        stop=True,
                    )
                    nc.tensor.matmul(
                        st[:, 1, :],
                        qkT[64:128, 2, ts(go, 128)],
                        qkT[64:128, 3, ts(tq, GQ)],
                        start=True,
                        stop=True,
                    )
                    est = estp.tile([128, 2, GQ], F32, tag="est")
                    nc.scalar.activation(
                        est[:], st[:], mybir.ActivationFunctionType.Exp, scale=0.125
                    )
                    nc.tensor.matmul(
                        ps_ot2[:],
                        vaug[:, ge, 2, :],
                        est[:, 0, :],
                        start=(g2 == 0),
                        stop=False,
                    )
                    nc.tensor.matmul(
                        ps_ot2[:],
                        vaug[:, go, 2, :],
                        est[:, 1, :],
                        start=False,
                        stop=(g2 == NTK // 2 - 1),
                    )
                normalize(ps_ot2, ot_tile[:, 2, :])

                # -- output projection for this q-block --
                for tsub in range(GQ // 128):
                    y_sb = yp.tile([128, C], F32, tag="y_sb")
                    for nh in range(2):
                        py = psY.tile([128, 384], F32, tag="y")
                        for h in range(HPC):
                            nc.tensor.matmul(
                                py[:],
                                ot_tile[:, h, ts(tsub, 128)],
                                wout_sb[:, h, ts(nh, 384)],
                                start=(h == 0),
                                stop=(h == HPC - 1),
                            )
                        nc.vector.tensor_tensor(
                            y_sb[:, ts(nh, 384)],
                            py[:],
                            bout_bc[:, ts(nh, 384)],
                            mybir.AluOpType.add,
                        )
                    nc.sync.dma_start(y3[:, tq * (GQ // 128) + tsub, :], y_sb[:])

    nc.compile()
    return nc


_PROGRAM = None


def _get_program():
    global _PROGRAM
    if _PROGRAM is None:
        _PROGRAM = _build_program()
    return _PROGRAM


def _make_in_maps(x, W_qkv, b_qkv, W_out, b_out):
    x = np.asarray(x, dtype=np.float32)
    W_qkv = np.asarray(W_qkv, dtype=np.float32)
    b_qkv = np.asarray(b_qkv, dtype=np.float32)
    W_out = np.asarray(W_out, dtype=np.float32)
    b_out = np.asarray(b_out, dtype=np.float32)

    xT_b = [np.ascontiguousarray(x[b].T) for b in range(B)]
    in_maps = []
    for c in range(N_CORES):
        b, g = divmod(c, 4)
        h0 = HPC * g

        def qcol(h):
            return slice(h * DK, (h + 1) * DK)

        def kcol(h):
            return slice(C + h * DK, C + (h + 1) * DK)

        wqk_c = np.concatenate(
            [
                W_qkv[:, qcol(h0)],
                W_qkv[:, qcol(h0 + 1)],
                W_qkv[:, kcol(h0)],
                W_qkv[:, kcol(h0 + 1)],
                W_qkv[:, qcol(h0 + 2)],
                W_qkv[:, kcol(h0 + 2)],
            ],
            axis=1,
        )
        bqk_c = np.concatenate(
            [
                b_qkv[qcol(h0)],
                b_qkv[qcol(h0 + 1)],
                b_qkv[kcol(h0)],
                b_qkv[kcol(h0 + 1)],
                b_qkv[qcol(h0 + 2)],
                b_qkv[kcol(h0 + 2)],
            ]
        )
        vs = slice(2 * C + h0 * DK, 2 * C + (h0 + HPC) * DK)
        in_maps.append(
            {
                "xT": np.ascontiguousarray(xT_b[b]),
                "wqk": np.ascontiguousarray(wqk_c),
                "bqk": np.ascontiguousarray(bqk_c),
                "wv": np.ascontiguousarray(W_qkv[:, vs]),
                "bv": np.ascontiguousarray(b_qkv[vs]),
                "wout": np.ascontiguousarray(W_out[h0 * DK : (h0 + HPC) * DK, :]),
                "bout": (b_out if g == 0 else np.zeros_like(b_out)).copy(),
            }
        )
    return in_maps


def _assemble(results):
    out = np.zeros((B, T, C), dtype=np.float32)
    for c in range(N_CORES):
        out[c // 4] += results[c]["y"]
    return out


def kernel_run(inputs, trace=False):
    """Returns (full_output [B,T,C] fp32, exec_time_ns or None)."""
    nc = _get_program()
    in_maps = _make_in_maps(**inputs)
    res = run_bass_kernel_spmd(
        nc, in_maps, core_ids=list(range(N_CORES)), trace=trace
    )
    return _assemble(res.results), res.exec_time_ns


def kernel(**inputs):
    out, _ = kernel_run(inputs)
    return out
